# revision 20
# baseline (speedup 1.0000x reference)
"""ChildSum TreeLSTM on 8 trn2 NeuronCores (Bass/Tile, SPMD feature-split).

Strategy
--------
head[j] > j, so the tree is topologically ordered. Nodes are relabeled
level-contiguously (leaves first). Hidden dim H=1024 is feature-split
across 8 cores (128 features each). Levels are processed in batches of
<=512 nodes; per batch:

  gates_p = sigmoid/tanh(Wx_p + sum_{k in ch(p)} (U g h_k))

Linearity: g_k = [U_i h_k; U_o h_k; U_u h_k; U_f h_k] (each core computes
its 4x128 slice) is computed once at k's own level, stored node-major in
DRAM; parents segment-sum gathered g rows with a one-hot S matmul on the
PE (lhsT = S). The forget path is nonlinear per child:
fc_p = sum_k sigmoid(Wxf_p + (U_f h_k)) * c_k, handled with gathered
rows + elementwise + the same S matmul.

Everything is node-major (nodes on PSUM partitions): one PSUM tile per
128-node subtile holds [i|o|u|Wxf] gate pre-activations (Wx accumulated
over 9 contraction chunks + segment-sum chunks), Wxf is sliced out to the
node-major wxf_store, c/h/g are written node-major with no transposes.
Only h is transposed (128x128 fp16 PE transposes) to build the
feature-major exchange slice.

Cross-core comm: each batch's h slice [128, bm] (fp16) is exchanged
SBUF->SBUF via gpsimd remote_dma_broadcast to all 8 cores (recv slot
indexed by sender partition id via an 8-way Switch), synchronized with
two alternating monotonic semaphores (+2 per sender per round at every
receiver; wait 16/round; round skew is <=1 because each core's next send
sits behind its own wait on the Pool engine). Each batch gets dedicated
send/recv tiles so no reuse handshake is needed.

All PE operands are fp16; PSUM accumulation and the c/h elementwise path
stay fp32.
"""
import numpy as np

N = 4096
H = 1024
HC = 128
NCORES = 8
PAD = N            # pad row index in node-major stores
BATCH = 512
CH = 128           # children per chunk
KCH = H // 128     # contraction chunks for U matmuls
KCHX = KCH + 1     # x contraction chunks incl. bias row
MAXNCH = 12


def _wrap_idx(a):
    """dma_gather index layout: idx[i] at [i%16, i//16], tiled to 128 partitions."""
    a = np.asarray(a, np.int64)
    n = len(a)
    c = (n + 15) // 16
    w = np.zeros((16, c), np.int16)
    w[np.arange(n) % 16, np.arange(n) // 16] = a.astype(np.int16)
    return np.tile(w, (8, 1))


def _schedule(head):
    head = np.asarray(head).astype(np.int64)
    n = head.shape[0]
    lev = np.zeros(n + 1, np.int64)
    for k in range(n):
        p = head[k]
        if lev[p] < lev[k] + 1:
            lev[p] = lev[k] + 1
    lv = lev[:n]
    order = np.argsort(lv, kind="stable")          # new -> old
    new_of_old = np.empty(n, np.int64)
    new_of_old[order] = np.arange(n)
    head_new = np.full(n, n, np.int64)
    for old in range(n):
        p = head[old]
        head_new[new_of_old[old]] = new_of_old[p] if p < n else n
    nlev = int(lv.max()) + 1
    mlev = [int((lv == L).sum()) for L in range(nlev)]
    start = np.concatenate([[0], np.cumsum(mlev)])
    kids = [[] for _ in range(n)]
    for k in range(n):
        p = head_new[k]
        if p < n:
            kids[p].append(k)

    batches = []
    for L in range(nlev):
        gs = int(start[L])
        while gs < start[L + 1]:
            bm = int(min(BATCH, start[L + 1] - gs))
            batches.append([L, gs, bm])
            gs += bm

    idx_blocks = []      # int16 wrapped blocks, concat on axis 1
    s_blocks = []        # [128, win] fp16 blocks
    icol = 0
    scol = 0
    binfos = []
    for (L, gs, bm) in batches:
        if L == 0:
            binfos.append(dict(L=L, gs=gs, bm=bm, chunks=[], nch=0))
            continue
        chunks = []      # (sub, plo_rel, win, s_off_rel) parent range within subtile
        slots_all = []
        wxf_all = []
        cur, curp = [], []
        plo = [None]
        phi = [None]

        def emit():
            padn = CH - len(cur)
            slots_all.extend(cur + [PAD] * padn)
            wxf_all.extend(curp + [PAD] * padn)
            sub = (plo[0] - gs) // 128
            S = np.zeros((CH, 128), np.float16)
            for s in range(len(curp)):
                S[s, curp[s] - gs - sub * 128] = 1.0
            chunks.append(sub)
            s_blocks.append(S)
            cur.clear()
            curp.clear()
            plo[0] = None

        for p in range(gs, gs + bm):
            ck = kids[p]
            assert 1 <= len(ck) <= CH
            # new chunk if slots full OR crossing a 128-node subtile boundary
            # (node-major PSUM: a chunk's parents live on one PSUM tile)
            if cur and (len(cur) + len(ck) > CH
                        or (p - gs) // 128 != (plo[0] - gs) // 128):
                emit()
            if plo[0] is None:
                plo[0] = p
            phi[0] = p
            cur.extend(ck)
            curp.extend([p] * len(ck))
        if cur:
            emit()
        nch = len(chunks)
        assert nch <= MAXNCH, nch
        wi = _wrap_idx(slots_all)
        ww = _wrap_idx(wxf_all)
        # per-chunk S col offsets (relative to this batch's scol)
        ch2 = []
        so = 0
        for sub in chunks:
            ch2.append((sub, so))
            so += 128
        binfos.append(dict(L=L, gs=gs, bm=bm, chunks=ch2, nch=nch,
                           icol_child=icol, icol_wxf=icol + wi.shape[1],
                           scol=scol, scols=so))
        idx_blocks.append(wi)
        idx_blocks.append(ww)
        icol += wi.shape[1] + ww.shape[1]
        scol += so

    idxt = (np.concatenate(idx_blocks, axis=1) if idx_blocks
            else np.zeros((128, 1), np.int16))
    sall = (np.concatenate(s_blocks, axis=1) if s_blocks
            else np.zeros((128, 1), np.float16))
    return dict(order=order, new_of_old=new_of_old, nlev=nlev,
                batches=binfos, idxt=idxt, sall=sall)


def _build_nc(sched):
    import concourse.mybir as mybir
    import concourse.tile as tile
    from concourse import bacc
    from concourse.masks import make_identity

    F32 = mybir.dt.float32
    F16 = mybir.dt.float16
    I16 = mybir.dt.int16
    SIG = mybir.ActivationFunctionType.Sigmoid
    TANH = mybir.ActivationFunctionType.Tanh

    binfos = sched["batches"]
    nlev = sched["nlev"]
    icols = sched["idxt"].shape[1]
    scols = sched["sall"].shape[1]

    nc = bacc.Bacc("TRN2", target_bir_lowering=False, debug=False,
                   num_devices=NCORES)
    xTf = nc.declare_dram_parameter("xTf", [KCHX * 128, N], F16, isOutput=False)
    WT4 = nc.declare_dram_parameter("WT4", [KCHX * 128, 512], F16, isOutput=False)
    UT = nc.declare_dram_parameter("UT", [H, 512], F16, isOutput=False)
    SALL = nc.declare_dram_parameter("SALL", [128, scols], F16, isOutput=False)
    IDXT = nc.declare_dram_parameter("IDXT", [128, icols], I16, isOutput=False)
    h_out = nc.declare_dram_parameter("h_out", [N, HC], F32, isOutput=True)
    c_out = nc.declare_dram_parameter("c_out", [N + 1, HC], F32, isOutput=True)

    g_store = nc.dram_tensor("g_store", [N + 1, 512], F16)
    wxf_store = nc.dram_tensor("wxf_store", [N + 1, HC], F16)
    ag_ins, ag_outs = [], []
    for bi, b in enumerate(binfos):
        if b["L"] == nlev - 1:
            ag_ins.append(None)
            ag_outs.append(None)
        else:
            ag_ins.append(nc.dram_tensor(f"agi{bi}", [128, b["bm"]], F16))
            ag_outs.append(nc.dram_tensor(f"ago{bi}", [H, b["bm"]], F16,
                                          addr_space="Shared"))

    ecnt = [0]

    def cpcopy(out, in_):
        ecnt[0] += 1
        if ecnt[0] % 2:
            nc.vector.tensor_copy(out, in_)
        else:
            nc.scalar.copy(out, in_)

    with tile.TileContext(nc) as tc:
        with (
            tc.tile_pool(name="const", bufs=1) as cpool,
            tc.tile_pool(name="xch", bufs=1) as xpool,
            tc.tile_pool(name="work", bufs=2) as wp,
            tc.tile_pool(name="gt", bufs=2) as gtp,
            tc.tile_pool(name="psA", bufs=1, space="PSUM") as psA,
            tc.tile_pool(name="psF", bufs=2, space="PSUM") as psF,
        ):
            identf16 = cpool.tile([128, 128], F16)
            make_identity(nc, identf16[:])
            wt4_sb = cpool.tile([128, KCHX, 512], F16)
            nc.sync.dma_start(wt4_sb[:], WT4[:].rearrange("(k p) j -> p k j", p=128))
            ut_sb = cpool.tile([128, KCH, 512], F16)
            nc.sync.dma_start(ut_sb[:], UT[:].rearrange("(k p) j -> p k j", p=128))
            idx_sb = cpool.tile([128, icols], I16)
            nc.sync.dma_start(idx_sb[:], IDXT[:])
            sall_sb = cpool.tile([128, scols], F16)
            nc.sync.dma_start(sall_sb[:], SALL[:])
            zrow16 = cpool.tile([1, 512], F16)
            nc.vector.memset(zrow16[:], 0.0)
            nc.sync.dma_start(g_store[N:N + 1, :], zrow16[:, :])
            nc.sync.dma_start(wxf_store[N:N + 1, :], zrow16[:, :HC])
            zrow32 = cpool.tile([1, HC], F32)
            nc.vector.memset(zrow32[:], 0.0)
            nc.sync.dma_start(c_out[N:N + 1, :], zrow32[:, :])

            round_no = [0]
            for bi, b in enumerate(binfos):
                L, gs, bm, nch = b["L"], b["gs"], b["bm"], b["nch"]
                last = (L == nlev - 1)
                nsub = (bm + 127) // 128
                if L > 0:
                    co = b["icol_child"]
                    wo = b["icol_wxf"]
                    ic = nch * 8
                    # merged gather of all 512 g cols (i,o,u blocks + Uf h)
                    gfull = gtp.tile([128, MAXNCH, 512], F16, tag="gfull")
                    nc.gpsimd.dma_gather(
                        out_ap=gfull[:, :nch, :], in_ap=g_store[:, :],
                        idxs_ap=idx_sb[:, co:co + ic],
                        num_idxs=nch * 128, num_idxs_reg=nch * 128,
                        elem_size=512)
                    gc = gtp.tile([128, MAXNCH, 128], F32, tag="gc")
                    nc.gpsimd.dma_gather(
                        out_ap=gc[:, :nch, :], in_ap=c_out[:, :],
                        idxs_ap=idx_sb[:, co:co + ic],
                        num_idxs=nch * 128, num_idxs_reg=nch * 128,
                        elem_size=128)
                xt_t = gtp.tile([128, KCHX, bm], F16, tag="xt")
                nc.sync.dma_start(
                    xt_t[:], xTf[:, gs:gs + bm]
                    .rearrange("(k p) j -> p k j", p=128))

                # ---- gates + epilogue, node-major, per 128-node subtile ----
                send_t = None
                if not last:
                    send_t = xpool.tile([128, bm], F16, tag="snd")
                h_nm = []
                c_nm = []
                wxf16 = []
                io_l, u_l = [], []
                for s in range(nsub):
                    sw = min(128, bm - s * 128)
                    p_t = psA.tile([128, 512], F32, tag=f"P{s % 3}",
                                   name=f"ps{bi}_{s}")
                    # gate pre-activations: Wx (all 4 gates incl. Wxf) + U hsum
                    for k in range(KCHX):
                        nc.tensor.matmul(
                            p_t[:sw, :], xt_t[:, k, s * 128:s * 128 + sw],
                            wt4_sb[:, k, :],
                            start=(k == 0), stop=(k == KCHX - 1))
                        if k == 0 and L > 0:
                            for cidx2, (sub, so) in enumerate(b["chunks"]):
                                if sub != s:
                                    continue
                                sAP = sall_sb[:, b["scol"] + so:
                                              b["scol"] + so + sw]
                                nc.tensor.matmul(
                                    p_t[:sw, 0:384], sAP,
                                    gfull[:, cidx2, 0:384],
                                    start=False, stop=False)
                    io_sb = wp.tile([128, 256], F32, tag="iosb", bufs=5)
                    nc.scalar.activation(io_sb[:sw, :], p_t[:sw, 0:256], SIG)
                    io_l.append(io_sb)
                    u_sb = wp.tile([128, 128], F32, tag="usb", bufs=5)
                    nc.scalar.activation(u_sb[:sw, :], p_t[:sw, 256:384], TANH)
                    u_l.append(u_sb)
                    w16 = wp.tile([128, 128], F16, tag="wxf16", bufs=5)
                    cpcopy(w16[:sw, :], p_t[:sw, 384:512])
                    wxf16.append(w16)
                    r0 = gs + s * 128
                    nc.sync.dma_start(wxf_store[r0:r0 + sw, :], w16[:sw, :])

                # Wxf rows of this batch are now stored; gather per child slot
                gw = None
                if L > 0:
                    gw = gtp.tile([128, MAXNCH, 128], F16, tag="gw")
                    nc.gpsimd.dma_gather(
                        out_ap=gw[:, :nch, :], in_ap=wxf_store[:, :],
                        idxs_ap=idx_sb[:, wo:wo + ic],
                        num_idxs=nch * 128, num_idxs_reg=nch * 128,
                        elem_size=128)

                for s in range(nsub):
                    sw = min(128, bm - s * 128)
                    f_t = None
                    if L > 0:
                        f_t = psF.tile([128, 128], F32, tag="PF",
                                       name=f"pf{bi}_{s}")
                        my_chunks = [ci for ci, (sub, so) in
                                     enumerate(b["chunks"]) if sub == s]
                        for j, cidx in enumerate(my_chunks):
                            so = b["chunks"][cidx][1]
                            sAP = sall_sb[:, b["scol"] + so:
                                          b["scol"] + so + sw]
                            t1 = wp.tile([128, 128], F16, tag="fc1")
                            nc.vector.tensor_add(t1[:],
                                                 gfull[:, cidx, 384:512],
                                                 gw[:, cidx, :])
                            t2 = wp.tile([128, 128], F32, tag="fc2")
                            nc.scalar.activation(t2[:], t1[:], SIG)
                            t3 = wp.tile([128, 128], F16, tag="fc3")
                            nc.vector.tensor_mul(t3[:], t2[:], gc[:, cidx, :])
                            nc.tensor.matmul(
                                f_t[:sw, :], sAP, t3[:],
                                start=(j == 0),
                                stop=(j == len(my_chunks) - 1))
                    c_t = wp.tile([128, 128], F32, tag="csb", bufs=5)
                    if L == 0:
                        nc.vector.tensor_mul(c_t[:sw, :], io_l[s][:sw, 0:128],
                                             u_l[s][:sw, :])
                    else:
                        tmp = wp.tile([128, 128], F32, tag="iu")
                        nc.vector.tensor_mul(tmp[:sw, :], io_l[s][:sw, 0:128],
                                             u_l[s][:sw, :])
                        nc.vector.tensor_add(c_t[:sw, :], tmp[:sw, :],
                                             f_t[:sw, :])
                    c_nm.append(c_t)
                    th = wp.tile([128, 128], F32, tag="thsb")
                    nc.scalar.activation(th[:sw, :], c_t[:sw, :], TANH)
                    h_t = wp.tile([128, 128], F32, tag="hsb", bufs=5)
                    nc.vector.tensor_mul(h_t[:sw, :], io_l[s][:sw, 128:256],
                                         th[:sw, :])
                    h_nm.append(h_t)
                    if not last:
                        h16 = wp.tile([128, 128], F16, tag="h16")
                        cpcopy(h16[:sw, :], h_t[:sw, :])
                        pt = psF.tile([128, 128], F16, tag="pth")
                        nc.tensor.transpose(pt[:, :sw], h16[:sw, :],
                                            identf16[:sw, :sw])
                        cpcopy(send_t[:, s * 128:s * 128 + sw], pt[:, :sw])

                # ---- exchange: AllGather of the fp16 h slice ----
                if not last:
                    nc.sync.dma_start(ag_ins[bi][:], send_t[:])
                    nc.gpsimd.collective_compute(
                        "AllGather", mybir.AluOpType.bypass,
                        replica_groups=[list(range(NCORES))],
                        ins=[ag_ins[bi][:]], outs=[ag_outs[bi][:]])
                    recv_t = xpool.tile([128, KCH, bm], F16, tag="rcv",
                                        bufs=2)
                    nc.sync.dma_start(
                        recv_t[:],
                        ag_outs[bi][:].rearrange("(k p) j -> p k j", p=128))

                # ---- stores (off the exchange critical path) ----
                for s in range(nsub):
                    sw = min(128, bm - s * 128)
                    r0 = gs + s * 128
                    nc.sync.dma_start(h_out[r0:r0 + sw, :], h_nm[s][:sw, :])
                    nc.sync.dma_start(c_out[r0:r0 + sw, :], c_nm[s][:sw, :])

                # ---- g = U @ h (full h from recv), node-major store ----
                if not last:
                    for s in range(nsub):
                        sw = min(128, bm - s * 128)
                        g_t = psA.tile([128, 512], F32, tag="G0",
                                       name=f"psg{bi}_{s}")
                        for k in range(KCH):
                            nc.tensor.matmul(
                                g_t[:sw, :],
                                recv_t[:, k, s * 128:s * 128 + sw],
                                ut_sb[:, k, :],
                                start=(k == 0), stop=(k == KCH - 1))
                        g16 = wp.tile([128, 512], F16, tag="g16")
                        cpcopy(g16[:sw, :], g_t[:sw, :])
                        r0 = gs + s * 128
                        nc.sync.dma_start(g_store[r0:r0 + sw, :],
                                          g16[:sw, :])

    nc.finalize()
    return nc


def build_for_sim(x=None, head=None, **kw):
    x = np.asarray(x, np.float32)
    head_np = np.asarray(head)
    sched = _schedule(head_np)
    order = sched["order"]
    new_of_old = sched["new_of_old"]

    n = x.shape[0]
    # xT padded with bias row at row H (ones), zeros after; columns in new order
    xTf = np.zeros((KCHX * 128, n), np.float16)
    xTf[:H, :] = x[order].T.astype(np.float16)
    xTf[H, :] = 1.0

    Ws = {g: np.asarray(kw[f"W_{g}"], np.float32) for g in "iouf"}
    Us = {g: np.asarray(kw[f"U_{g}"], np.float32) for g in "iouf"}
    bs = {g: np.asarray(kw[f"b_{g}"], np.float32) for g in "iouf"}

    in_maps = []
    for c in range(NCORES):
        sl = slice(c * HC, (c + 1) * HC)
        WT4 = np.zeros((KCHX * 128, 512), np.float16)
        UT = np.zeros((H, 512), np.float16)
        for gi_, g in enumerate("iouf"):
            WT4[:H, gi_ * 128:(gi_ + 1) * 128] = Ws[g][sl, :].T
            WT4[H, gi_ * 128:(gi_ + 1) * 128] = bs[g][sl]
            UT[:, gi_ * 128:(gi_ + 1) * 128] = Us[g][sl, :].T
        in_maps.append({
            "xTf": xTf, "WT4": WT4, "UT": UT,
            "SALL": np.ascontiguousarray(sched["sall"]),
            "IDXT": np.ascontiguousarray(sched["idxt"]),
        })

    nc = _build_nc(sched)

    def unshard(results):
        h_new = np.concatenate([results[c]["h_out"] for c in range(NCORES)],
                               axis=1)
        c_new = np.concatenate([results[c]["c_out"][:n] for c in range(NCORES)],
                               axis=1)
        return h_new[new_of_old], c_new[new_of_old]

    post = {"out_names": ["h_out", "c_out"], "unshard": unshard}
    return nc, in_maps, post


def kernel(x=None, head=None, **kw):
    import concourse.mybir as mybir  # noqa: F401  (env check)
    from concourse.bass_utils import run_bass_kernel_spmd

    nc, in_maps, post = build_for_sim(x=x, head=head, **kw)
    res = run_bass_kernel_spmd(nc, in_maps, list(range(NCORES)))
    return post["unshard"](res.results)


# revision 21
# speedup vs baseline: 1.0039x; 1.0039x over previous
"""ChildSum TreeLSTM on 8 trn2 NeuronCores (Bass/Tile, SPMD feature-split).

Strategy
--------
head[j] > j, so the tree is topologically ordered. Nodes are relabeled
level-contiguously (leaves first). Hidden dim H=1024 is feature-split
across 8 cores (128 features each). Levels are processed in batches of
<=512 nodes; per batch:

  gates_p = sigmoid/tanh(Wx_p + sum_{k in ch(p)} (U g h_k))

Linearity: g_k = [U_i h_k; U_o h_k; U_u h_k; U_f h_k] (each core computes
its 4x128 slice) is computed once at k's own level, stored node-major in
DRAM; parents segment-sum gathered g rows with a one-hot S matmul on the
PE (lhsT = S). The forget path is nonlinear per child:
fc_p = sum_k sigmoid(Wxf_p + (U_f h_k)) * c_k, handled with gathered
rows + elementwise + the same S matmul.

Everything is node-major (nodes on PSUM partitions): one PSUM tile per
128-node subtile holds [i|o|u|Wxf] gate pre-activations (Wx accumulated
over 9 contraction chunks + segment-sum chunks), Wxf is sliced out to the
node-major wxf_store, c/h/g are written node-major with no transposes.
Only h is transposed (128x128 fp16 PE transposes) to build the
feature-major exchange slice.

Cross-core comm: each batch's h slice [128, bm] (fp16) is exchanged
SBUF->SBUF via gpsimd remote_dma_broadcast to all 8 cores (recv slot
indexed by sender partition id via an 8-way Switch), synchronized with
two alternating monotonic semaphores (+2 per sender per round at every
receiver; wait 16/round; round skew is <=1 because each core's next send
sits behind its own wait on the Pool engine). Each batch gets dedicated
send/recv tiles so no reuse handshake is needed.

All PE operands are fp16; PSUM accumulation and the c/h elementwise path
stay fp32.
"""
import numpy as np

N = 4096
H = 1024
HC = 128
NCORES = 8
PAD = N            # pad row index in node-major stores
BATCH = 512
CH = 128           # children per chunk
KCH = H // 128     # contraction chunks for U matmuls
KCHX = KCH + 1     # x contraction chunks incl. bias row
MAXNCH = 12


def _wrap_idx(a):
    """dma_gather index layout: idx[i] at [i%16, i//16], tiled to 128 partitions."""
    a = np.asarray(a, np.int64)
    n = len(a)
    c = (n + 15) // 16
    w = np.zeros((16, c), np.int16)
    w[np.arange(n) % 16, np.arange(n) // 16] = a.astype(np.int16)
    return np.tile(w, (8, 1))


def _schedule(head):
    head = np.asarray(head).astype(np.int64)
    n = head.shape[0]
    lev = np.zeros(n + 1, np.int64)
    for k in range(n):
        p = head[k]
        if lev[p] < lev[k] + 1:
            lev[p] = lev[k] + 1
    lv = lev[:n]
    order = np.argsort(lv, kind="stable")          # new -> old
    new_of_old = np.empty(n, np.int64)
    new_of_old[order] = np.arange(n)
    head_new = np.full(n, n, np.int64)
    for old in range(n):
        p = head[old]
        head_new[new_of_old[old]] = new_of_old[p] if p < n else n
    nlev = int(lv.max()) + 1
    mlev = [int((lv == L).sum()) for L in range(nlev)]
    start = np.concatenate([[0], np.cumsum(mlev)])
    kids = [[] for _ in range(n)]
    for k in range(n):
        p = head_new[k]
        if p < n:
            kids[p].append(k)

    batches = []
    for L in range(nlev):
        gs = int(start[L])
        while gs < start[L + 1]:
            bm = int(min(BATCH, start[L + 1] - gs))
            batches.append([L, gs, bm])
            gs += bm

    idx_blocks = []      # int16 wrapped blocks, concat on axis 1
    s_blocks = []        # [128, win] fp16 blocks
    icol = 0
    scol = 0
    binfos = []
    for (L, gs, bm) in batches:
        if L == 0:
            binfos.append(dict(L=L, gs=gs, bm=bm, chunks=[], nch=0))
            continue
        chunks = []      # (sub, plo_rel, win, s_off_rel) parent range within subtile
        slots_all = []
        wxf_all = []
        cur, curp = [], []
        plo = [None]
        phi = [None]

        def emit():
            padn = CH - len(cur)
            slots_all.extend(cur + [PAD] * padn)
            wxf_all.extend(curp + [PAD] * padn)
            sub = (plo[0] - gs) // 128
            S = np.zeros((CH, 128), np.float16)
            for s in range(len(curp)):
                S[s, curp[s] - gs - sub * 128] = 1.0
            chunks.append(sub)
            s_blocks.append(S)
            cur.clear()
            curp.clear()
            plo[0] = None

        for p in range(gs, gs + bm):
            ck = kids[p]
            assert 1 <= len(ck) <= CH
            # new chunk if slots full OR crossing a 128-node subtile boundary
            # (node-major PSUM: a chunk's parents live on one PSUM tile)
            if cur and (len(cur) + len(ck) > CH
                        or (p - gs) // 128 != (plo[0] - gs) // 128):
                emit()
            if plo[0] is None:
                plo[0] = p
            phi[0] = p
            cur.extend(ck)
            curp.extend([p] * len(ck))
        if cur:
            emit()
        nch = len(chunks)
        assert nch <= MAXNCH, nch
        wi = _wrap_idx(slots_all)
        ww = _wrap_idx(wxf_all)
        # per-chunk S col offsets (relative to this batch's scol)
        ch2 = []
        so = 0
        for sub in chunks:
            ch2.append((sub, so))
            so += 128
        binfos.append(dict(L=L, gs=gs, bm=bm, chunks=ch2, nch=nch,
                           icol_child=icol, icol_wxf=icol + wi.shape[1],
                           scol=scol, scols=so))
        idx_blocks.append(wi)
        idx_blocks.append(ww)
        icol += wi.shape[1] + ww.shape[1]
        scol += so

    idxt = (np.concatenate(idx_blocks, axis=1) if idx_blocks
            else np.zeros((128, 1), np.int16))
    sall = (np.concatenate(s_blocks, axis=1) if s_blocks
            else np.zeros((128, 1), np.float16))
    return dict(order=order, new_of_old=new_of_old, nlev=nlev,
                batches=binfos, idxt=idxt, sall=sall)


def _build_nc(sched):
    import concourse.mybir as mybir
    import concourse.tile as tile
    from concourse import bacc
    from concourse.masks import make_identity

    F32 = mybir.dt.float32
    F16 = mybir.dt.float16
    I16 = mybir.dt.int16
    SIG = mybir.ActivationFunctionType.Sigmoid
    TANH = mybir.ActivationFunctionType.Tanh

    binfos = sched["batches"]
    nlev = sched["nlev"]
    icols = sched["idxt"].shape[1]
    scols = sched["sall"].shape[1]

    nc = bacc.Bacc("TRN2", target_bir_lowering=False, debug=False,
                   num_devices=NCORES)
    xTf = nc.declare_dram_parameter("xTf", [KCHX * 128, N], F16, isOutput=False)
    WT4 = nc.declare_dram_parameter("WT4", [KCHX * 128, 512], F16, isOutput=False)
    UT = nc.declare_dram_parameter("UT", [H, 512], F16, isOutput=False)
    SALL = nc.declare_dram_parameter("SALL", [128, scols], F16, isOutput=False)
    IDXT = nc.declare_dram_parameter("IDXT", [128, icols], I16, isOutput=False)
    h_out = nc.declare_dram_parameter("h_out", [N, HC], F32, isOutput=True)
    c_out = nc.declare_dram_parameter("c_out", [N + 1, HC], F32, isOutput=True)

    g_store = nc.dram_tensor("g_store", [N + 1, 512], F16)
    wxf_store = nc.dram_tensor("wxf_store", [N + 1, HC], F16)
    lev_start, lev_m = {}, {}
    for b in binfos:
        L = b["L"]
        if L not in lev_start:
            lev_start[L] = b["gs"]
            lev_m[L] = 0
        lev_m[L] += b["bm"]
    ag_ins, ag_outs = {}, {}
    for L in range(nlev - 1):
        ag_ins[L] = nc.dram_tensor(f"agi{L}", [128, lev_m[L]], F16)
        ag_outs[L] = nc.dram_tensor(f"ago{L}", [H, lev_m[L]], F16,
                                    addr_space="Shared")

    ecnt = [0]

    def cpcopy(out, in_):
        ecnt[0] += 1
        if ecnt[0] % 2:
            nc.vector.tensor_copy(out, in_)
        else:
            nc.scalar.copy(out, in_)

    with tile.TileContext(nc) as tc:
        with (
            tc.tile_pool(name="const", bufs=1) as cpool,
            tc.tile_pool(name="xch", bufs=1) as xpool,
            tc.tile_pool(name="work", bufs=2) as wp,
            tc.tile_pool(name="gt", bufs=2) as gtp,
            tc.tile_pool(name="psA", bufs=1, space="PSUM") as psA,
            tc.tile_pool(name="psF", bufs=2, space="PSUM") as psF,
        ):
            identf16 = cpool.tile([128, 128], F16)
            make_identity(nc, identf16[:])
            wt4_sb = cpool.tile([128, KCHX, 512], F16)
            nc.sync.dma_start(wt4_sb[:], WT4[:].rearrange("(k p) j -> p k j", p=128))
            ut_sb = cpool.tile([128, KCH, 512], F16)
            nc.sync.dma_start(ut_sb[:], UT[:].rearrange("(k p) j -> p k j", p=128))
            idx_sb = cpool.tile([128, icols], I16)
            nc.sync.dma_start(idx_sb[:], IDXT[:])
            sall_sb = cpool.tile([128, scols], F16)
            nc.sync.dma_start(sall_sb[:], SALL[:])
            zrow16 = cpool.tile([1, 512], F16)
            nc.vector.memset(zrow16[:], 0.0)
            nc.sync.dma_start(g_store[N:N + 1, :], zrow16[:, :])
            nc.sync.dma_start(wxf_store[N:N + 1, :], zrow16[:, :HC])
            zrow32 = cpool.tile([1, HC], F32)
            nc.vector.memset(zrow32[:], 0.0)
            nc.sync.dma_start(c_out[N:N + 1, :], zrow32[:, :])

            round_no = [0]
            for bi, b in enumerate(binfos):
                L, gs, bm, nch = b["L"], b["gs"], b["bm"], b["nch"]
                last = (L == nlev - 1)
                nsub = (bm + 127) // 128
                if L > 0:
                    co = b["icol_child"]
                    wo = b["icol_wxf"]
                    ic = nch * 8
                    # merged gather of all 512 g cols (i,o,u blocks + Uf h)
                    gfull = gtp.tile([128, MAXNCH, 512], F16, tag="gfull")
                    nc.gpsimd.dma_gather(
                        out_ap=gfull[:, :nch, :], in_ap=g_store[:, :],
                        idxs_ap=idx_sb[:, co:co + ic],
                        num_idxs=nch * 128, num_idxs_reg=nch * 128,
                        elem_size=512)
                    gc = gtp.tile([128, MAXNCH, 128], F32, tag="gc")
                    nc.gpsimd.dma_gather(
                        out_ap=gc[:, :nch, :], in_ap=c_out[:, :],
                        idxs_ap=idx_sb[:, co:co + ic],
                        num_idxs=nch * 128, num_idxs_reg=nch * 128,
                        elem_size=128)
                xt_t = gtp.tile([128, KCHX, bm], F16, tag="xt")
                nc.sync.dma_start(
                    xt_t[:], xTf[:, gs:gs + bm]
                    .rearrange("(k p) j -> p k j", p=128))

                # ---- gates + epilogue, node-major, per 128-node subtile ----
                send_t = None
                if not last:
                    send_t = xpool.tile([128, bm], F16, tag="snd")
                h_nm = []
                c_nm = []
                wxf16 = []
                io_l, u_l = [], []
                for s in range(nsub):
                    sw = min(128, bm - s * 128)
                    p_t = psA.tile([128, 512], F32, tag=f"P{s % 3}",
                                   name=f"ps{bi}_{s}")
                    # gate pre-activations: Wx (all 4 gates incl. Wxf) + U hsum
                    for k in range(KCHX):
                        nc.tensor.matmul(
                            p_t[:sw, :], xt_t[:, k, s * 128:s * 128 + sw],
                            wt4_sb[:, k, :],
                            start=(k == 0), stop=(k == KCHX - 1))
                        if k == 0 and L > 0:
                            for cidx2, (sub, so) in enumerate(b["chunks"]):
                                if sub != s:
                                    continue
                                sAP = sall_sb[:, b["scol"] + so:
                                              b["scol"] + so + sw]
                                nc.tensor.matmul(
                                    p_t[:sw, 0:384], sAP,
                                    gfull[:, cidx2, 0:384],
                                    start=False, stop=False)
                    io_sb = wp.tile([128, 256], F32, tag="iosb", bufs=5)
                    nc.scalar.activation(io_sb[:sw, :], p_t[:sw, 0:256], SIG)
                    io_l.append(io_sb)
                    u_sb = wp.tile([128, 128], F32, tag="usb", bufs=5)
                    nc.scalar.activation(u_sb[:sw, :], p_t[:sw, 256:384], TANH)
                    u_l.append(u_sb)
                    w16 = wp.tile([128, 128], F16, tag="wxf16", bufs=5)
                    cpcopy(w16[:sw, :], p_t[:sw, 384:512])
                    wxf16.append(w16)
                    r0 = gs + s * 128
                    nc.sync.dma_start(wxf_store[r0:r0 + sw, :], w16[:sw, :])

                # Wxf rows of this batch are now stored; gather per child slot
                gw = None
                if L > 0:
                    gw = gtp.tile([128, MAXNCH, 128], F16, tag="gw")
                    nc.gpsimd.dma_gather(
                        out_ap=gw[:, :nch, :], in_ap=wxf_store[:, :],
                        idxs_ap=idx_sb[:, wo:wo + ic],
                        num_idxs=nch * 128, num_idxs_reg=nch * 128,
                        elem_size=128)

                for s in range(nsub):
                    sw = min(128, bm - s * 128)
                    f_t = None
                    if L > 0:
                        f_t = psF.tile([128, 128], F32, tag="PF",
                                       name=f"pf{bi}_{s}")
                        my_chunks = [ci for ci, (sub, so) in
                                     enumerate(b["chunks"]) if sub == s]
                        for j, cidx in enumerate(my_chunks):
                            so = b["chunks"][cidx][1]
                            sAP = sall_sb[:, b["scol"] + so:
                                          b["scol"] + so + sw]
                            t1 = wp.tile([128, 128], F16, tag="fc1")
                            nc.vector.tensor_add(t1[:],
                                                 gfull[:, cidx, 384:512],
                                                 gw[:, cidx, :])
                            t2 = wp.tile([128, 128], F32, tag="fc2")
                            nc.scalar.activation(t2[:], t1[:], SIG)
                            t3 = wp.tile([128, 128], F16, tag="fc3")
                            nc.vector.tensor_mul(t3[:], t2[:], gc[:, cidx, :])
                            nc.tensor.matmul(
                                f_t[:sw, :], sAP, t3[:],
                                start=(j == 0),
                                stop=(j == len(my_chunks) - 1))
                    c_t = wp.tile([128, 128], F32, tag="csb", bufs=5)
                    if L == 0:
                        nc.vector.tensor_mul(c_t[:sw, :], io_l[s][:sw, 0:128],
                                             u_l[s][:sw, :])
                    else:
                        tmp = wp.tile([128, 128], F32, tag="iu")
                        nc.vector.tensor_mul(tmp[:sw, :], io_l[s][:sw, 0:128],
                                             u_l[s][:sw, :])
                        nc.vector.tensor_add(c_t[:sw, :], tmp[:sw, :],
                                             f_t[:sw, :])
                    c_nm.append(c_t)
                    th = wp.tile([128, 128], F32, tag="thsb")
                    nc.scalar.activation(th[:sw, :], c_t[:sw, :], TANH)
                    h_t = wp.tile([128, 128], F32, tag="hsb", bufs=5)
                    nc.vector.tensor_mul(h_t[:sw, :], io_l[s][:sw, 128:256],
                                         th[:sw, :])
                    h_nm.append(h_t)
                    if not last:
                        h16 = wp.tile([128, 128], F16, tag="h16")
                        cpcopy(h16[:sw, :], h_t[:sw, :])
                        pt = psF.tile([128, 128], F16, tag="pth")
                        nc.tensor.transpose(pt[:, :sw], h16[:sw, :],
                                            identf16[:sw, :sw])
                        cpcopy(send_t[:, s * 128:s * 128 + sw], pt[:, :sw])

                # ---- stage this batch's h slice into the level AG input ----
                if not last:
                    ls = lev_start[L]
                    nc.sync.dma_start(
                        ag_ins[L][:, gs - ls:gs - ls + bm], send_t[:])

                # ---- stores (off the exchange critical path) ----
                for s in range(nsub):
                    sw = min(128, bm - s * 128)
                    r0 = gs + s * 128
                    nc.sync.dma_start(h_out[r0:r0 + sw, :], h_nm[s][:sw, :])
                    nc.sync.dma_start(c_out[r0:r0 + sw, :], c_nm[s][:sw, :])

                # ---- level epilog: one AllGather + g = U @ h, node-major ----
                level_done = (bi + 1 == len(binfos)
                              or binfos[bi + 1]["L"] != L)
                if not last and level_done:
                    ls = lev_start[L]
                    ml = lev_m[L]
                    nc.gpsimd.collective_compute(
                        "AllGather", mybir.AluOpType.bypass,
                        replica_groups=[list(range(NCORES))],
                        ins=[ag_ins[L][:]], outs=[ag_outs[L][:]])
                    recv_t = xpool.tile([128, KCH, ml], F16, tag="rcv",
                                        bufs=1)
                    nc.sync.dma_start(
                        recv_t[:],
                        ag_outs[L][:].rearrange("(k p) j -> p k j", p=128))
                    for s in range((ml + 127) // 128):
                        sw = min(128, ml - s * 128)
                        g_t = psA.tile([128, 512], F32, tag="G0",
                                       name=f"psg{L}_{s}")
                        for k in range(KCH):
                            nc.tensor.matmul(
                                g_t[:sw, :],
                                recv_t[:, k, s * 128:s * 128 + sw],
                                ut_sb[:, k, :],
                                start=(k == 0), stop=(k == KCH - 1))
                        g16 = wp.tile([128, 512], F16, tag="g16")
                        cpcopy(g16[:sw, :], g_t[:sw, :])
                        r0 = ls + s * 128
                        nc.sync.dma_start(g_store[r0:r0 + sw, :],
                                          g16[:sw, :])

    nc.finalize()
    return nc


def build_for_sim(x=None, head=None, **kw):
    x = np.asarray(x, np.float32)
    head_np = np.asarray(head)
    sched = _schedule(head_np)
    order = sched["order"]
    new_of_old = sched["new_of_old"]

    n = x.shape[0]
    # xT padded with bias row at row H (ones), zeros after; columns in new order
    xTf = np.zeros((KCHX * 128, n), np.float16)
    xTf[:H, :] = x[order].T.astype(np.float16)
    xTf[H, :] = 1.0

    Ws = {g: np.asarray(kw[f"W_{g}"], np.float32) for g in "iouf"}
    Us = {g: np.asarray(kw[f"U_{g}"], np.float32) for g in "iouf"}
    bs = {g: np.asarray(kw[f"b_{g}"], np.float32) for g in "iouf"}

    in_maps = []
    for c in range(NCORES):
        sl = slice(c * HC, (c + 1) * HC)
        WT4 = np.zeros((KCHX * 128, 512), np.float16)
        UT = np.zeros((H, 512), np.float16)
        for gi_, g in enumerate("iouf"):
            WT4[:H, gi_ * 128:(gi_ + 1) * 128] = Ws[g][sl, :].T
            WT4[H, gi_ * 128:(gi_ + 1) * 128] = bs[g][sl]
            UT[:, gi_ * 128:(gi_ + 1) * 128] = Us[g][sl, :].T
        in_maps.append({
            "xTf": xTf, "WT4": WT4, "UT": UT,
            "SALL": np.ascontiguousarray(sched["sall"]),
            "IDXT": np.ascontiguousarray(sched["idxt"]),
        })

    nc = _build_nc(sched)

    def unshard(results):
        h_new = np.concatenate([results[c]["h_out"] for c in range(NCORES)],
                               axis=1)
        c_new = np.concatenate([results[c]["c_out"][:n] for c in range(NCORES)],
                               axis=1)
        return h_new[new_of_old], c_new[new_of_old]

    post = {"out_names": ["h_out", "c_out"], "unshard": unshard}
    return nc, in_maps, post


def kernel(x=None, head=None, **kw):
    import concourse.mybir as mybir  # noqa: F401  (env check)
    from concourse.bass_utils import run_bass_kernel_spmd

    nc, in_maps, post = build_for_sim(x=x, head=head, **kw)
    res = run_bass_kernel_spmd(nc, in_maps, list(range(NCORES)))
    return post["unshard"](res.results)
